# revision 1
# baseline (speedup 1.0000x reference)
"""Trainium2 Bass kernel for the MoE routing problem.

Strategy (expert-parallel, host dispatch/combine):
  - Router runs on host in float64 (top-2 selection, softmax weights,
    aux load-balancing loss) — negligible FLOPs.
  - Core e computes its expert's swiglu over the tokens routed to it
    (capacity-padded so all 8 cores run one SPMD program), plus a 1/8
    token-slice of the shared expert.
  - Activations/weights are cast to bf16 on host; matmuls accumulate in
    fp32 PSUM; outputs return fp32; combine (routing-weight scatter-add)
    happens on host in fp32.

Device layout: everything transposed ([D, T]) so the matmul contraction
dim is always the SBUF partition dim and no on-device transposes are
needed (gate/up/down weight matrices are all naturally contraction-major).
"""

import os
import sys

import numpy as np

for _p in ("/opt/trn_rl_repo",):
    if _p not in sys.path and os.path.isdir(_p):
        sys.path.insert(0, _p)

import ml_dtypes

import concourse.bass as bass
import concourse.mybir as mybir
import concourse.tile as tile

BF16_NP = ml_dtypes.bfloat16

D = 1024
I = 2048
P = 128
KD = D // P  # 8
KI = I // P  # 16
NUM_EXPERTS = 8
TOP_K = 2
N_CORES = 8
BF16 = mybir.dt.bfloat16
F32 = mybir.dt.float32
AF = mybir.ActivationFunctionType

_CACHE = {}


def _patch_tile_drain():
    """This walrus build accepts only one sync-wait per TPB_CTRL
    instruction; split the Tile tail drain's waits across standalone
    wait_ge instructions."""
    from concourse.tile import ScopedClock, TileContext

    if getattr(TileContext, "_moe_drain_patched", False):
        return

    def _patched(self, tick_clock, wait_clock):
        nc = self.nc
        drain_inst = nc.sync.drain()
        wait_clock.add_sem_waits(
            drain_inst.ins, ScopedClock({None: tick_clock.global_clock})
        )
        si = drain_inst.ins.sync_info
        waits = list(si.on_wait) if si is not None else []
        if len(waits) > 1:
            si.on_wait = waits[:1]
            handles = {}
            for name, h in self.sems.allocated().items():
                handles[getattr(h, "name", name)] = h
                handles[getattr(h, "num", None)] = h
            for w in waits[1:]:
                h = handles.get(w.ant_name) or handles.get(w.id)
                assert h is not None, f"no sem handle for {w.ant_name}"
                nc.sync.wait_ge(h, w.wait_value)
        nc.all_engine_barrier()
        popped = nc._tile_sem_poison_stack.pop()
        assert popped is self._sem_poison
        nc.clear_and_free_semaphores(list(self.sems.allocated().values()))
        nc.all_engine_barrier()

    TileContext._drain_and_barrier = _patched
    TileContext._moe_drain_patched = True


def _build_moe_nc(C_E: int, C_S: int, TB: int = 512, fused_silu: bool = True):
    assert C_E % P == 0 and C_S % P == 0
    nc = bass.Bass()

    xT_e = nc.declare_dram_parameter("xT_e", [D, C_E], BF16, isOutput=False)
    xT_s = nc.declare_dram_parameter("xT_s", [D, C_S], BF16, isOutput=False)
    params = {}
    for pfx in ("e", "s"):
        params[f"wg_{pfx}"] = nc.declare_dram_parameter(f"wg_{pfx}", [D, I], BF16, isOutput=False)
        params[f"wu_{pfx}"] = nc.declare_dram_parameter(f"wu_{pfx}", [D, I], BF16, isOutput=False)
        params[f"wd_{pfx}"] = nc.declare_dram_parameter(f"wd_{pfx}", [I, D], BF16, isOutput=False)
    yT_e = nc.declare_dram_parameter("yT_e", [D, C_E], F32, isOutput=True)
    yT_s = nc.declare_dram_parameter("yT_s", [D, C_S], F32, isOutput=True)

    with tile.TileContext(nc) as tc:
        with (
            tc.tile_pool(name="w", bufs=1) as wpool,
            tc.tile_pool(name="x", bufs=2) as xpool,
            tc.tile_pool(name="h", bufs=2) as hpool,
            tc.tile_pool(name="a", bufs=4) as apool,
            tc.tile_pool(name="zs", bufs=4) as zpool,
            tc.tile_pool(name="pgu", bufs=2, space="PSUM") as pgu,
            tc.tile_pool(name="pz", bufs=2, space="PSUM") as pzp,
        ):
            for pfx, xT, yT, C in (("e", xT_e, yT_e, C_E), ("s", xT_s, yT_s, C_S)):
                wg_sb = wpool.tile([P, KD * I], BF16, tag="wg", name=f"wg_sb_{pfx}")
                wu_sb = wpool.tile([P, KD * I], BF16, tag="wu", name=f"wu_sb_{pfx}")
                wd_sb = wpool.tile([P, KI * D], BF16, tag="wd", name=f"wd_sb_{pfx}")
                for k in range(KD):
                    nc.sync.dma_start(out=wg_sb[:, k * I:(k + 1) * I], in_=params[f"wg_{pfx}"][k * P:(k + 1) * P, :])
                    nc.sync.dma_start(out=wu_sb[:, k * I:(k + 1) * I], in_=params[f"wu_{pfx}"][k * P:(k + 1) * P, :])
                for k in range(KI):
                    nc.sync.dma_start(out=wd_sb[:, k * D:(k + 1) * D], in_=params[f"wd_{pfx}"][k * P:(k + 1) * P, :])

                for t0 in range(0, C, TB):
                    tb = min(TB, C - t0)
                    x_sb = xpool.tile([P, KD * TB], BF16, tag="x", name=f"x_{pfx}_{t0}")
                    for k in range(KD):
                        nc.sync.dma_start(out=x_sb[:, k * TB:k * TB + tb], in_=xT[k * P:(k + 1) * P, t0:t0 + tb])
                    h_sb = hpool.tile([P, KI * TB], BF16, tag="h", name=f"h_{pfx}_{t0}")
                    for it in range(KI):
                        pg = pgu.tile([P, TB], F32, tag="pg", name=f"pg_{pfx}_{t0}_{it}")
                        pu = pgu.tile([P, TB], F32, tag="pu", name=f"pu_{pfx}_{t0}_{it}")
                        for k in range(KD):
                            nc.tensor.matmul(
                                pg[:, :tb],
                                wg_sb[:, k * I + it * P: k * I + it * P + P],
                                x_sb[:, k * TB:k * TB + tb],
                                start=(k == 0), stop=(k == KD - 1),
                            )
                        for k in range(KD):
                            nc.tensor.matmul(
                                pu[:, :tb],
                                wu_sb[:, k * I + it * P: k * I + it * P + P],
                                x_sb[:, k * TB:k * TB + tb],
                                start=(k == 0), stop=(k == KD - 1),
                            )
                        h_slice = h_sb[:, it * TB:it * TB + tb]
                        s_sb = apool.tile([P, TB], F32, tag="s", name=f"s_{pfx}_{t0}_{it}")
                        if fused_silu:
                            nc.scalar.activation(s_sb[:, :tb], pg[:, :tb], AF.Silu)
                            nc.vector.tensor_mul(h_slice, pu[:, :tb], s_sb[:, :tb])
                        else:
                            nc.scalar.activation(s_sb[:, :tb], pg[:, :tb], AF.Sigmoid)
                            t_sb = apool.tile([P, TB], F32, tag="t", name=f"t_{pfx}_{t0}_{it}")
                            nc.vector.tensor_mul(t_sb[:, :tb], pg[:, :tb], s_sb[:, :tb])
                            nc.vector.tensor_mul(h_slice, pu[:, :tb], t_sb[:, :tb])
                    for dt in range(KD):
                        pz = pzp.tile([P, TB], F32, tag="pz", name=f"pz_{pfx}_{t0}_{dt}")
                        for it in range(KI):
                            nc.tensor.matmul(
                                pz[:, :tb],
                                wd_sb[:, it * D + dt * P: it * D + dt * P + P],
                                h_sb[:, it * TB:it * TB + tb],
                                start=(it == 0), stop=(it == KI - 1),
                            )
                        z_sb = zpool.tile([P, TB], F32, tag="z", name=f"z_{pfx}_{t0}_{dt}")
                        nc.vector.tensor_copy(out=z_sb[:, :tb], in_=pz[:, :tb])
                        nc.sync.dma_start(out=yT[dt * P:(dt + 1) * P, t0:t0 + tb], in_=z_sb[:, :tb])
    return nc


def _ensure_ntff_hook():
    """The agent image's antenv lacks axon_hooks; synthesize it and
    register the ctypes NTFF profiling hook so trace=True works."""
    import types
    try:
        from antenv.axon_hooks import get_axon_ntff_profile_hook  # noqa: F401
        return
    except ImportError:
        pass
    mod = types.ModuleType("antenv.axon_hooks")
    mod._hook = None

    def set_axon_ntff_profile_hook(h):
        mod._hook = h

    def get_axon_ntff_profile_hook():
        return mod._hook

    mod.set_axon_ntff_profile_hook = set_axon_ntff_profile_hook
    mod.get_axon_ntff_profile_hook = get_axon_ntff_profile_hook
    sys.modules["antenv.axon_hooks"] = mod
    try:
        from trn_agent_boot.trn_boot import _ntff_profile_via_ctypes
        h = _ntff_profile_via_ctypes("/opt/axon/libaxon_pjrt.so")
        if h is not None:
            set_axon_ntff_profile_hook(h)
    except Exception as e:
        print(f"ntff hook setup failed: {e}", file=sys.stderr)


def _route(x, gate_w):
    """float64 routing: top-2 selection, softmax weights, aux loss."""
    T = x.shape[0]
    lg = x.astype(np.float64) @ gate_w.astype(np.float64).T  # [T, E]
    sel = np.argsort(-lg, axis=-1, kind="stable")[:, :TOP_K]  # [T, 2]
    top_vals = np.take_along_axis(lg, sel, axis=1)
    ex = np.exp(top_vals - top_vals.max(axis=1, keepdims=True))
    w = ex / ex.sum(axis=1, keepdims=True)  # [T, 2]

    counts = np.zeros(NUM_EXPERTS, dtype=np.int64)
    np.add.at(counts, sel.ravel(), 1)
    f = counts / (T * TOP_K)
    el = np.exp(lg - lg.max(axis=1, keepdims=True))
    Pm = (el / el.sum(axis=1, keepdims=True)).mean(axis=0)
    aux = np.float32(NUM_EXPERTS * np.sum(f * Pm))
    return sel, w.astype(np.float32), counts, aux


def kernel(hidden_states, gate_w, shared_gate, shared_up, shared_down,
           exp_gate, exp_up, exp_down):
    from concourse.bass_utils import run_bass_kernel_spmd

    B, S, Dm = hidden_states.shape
    x = np.ascontiguousarray(np.asarray(hidden_states, dtype=np.float32).reshape(-1, Dm))
    T = x.shape[0]

    sel, w, counts, aux = _route(x, np.asarray(gate_w, dtype=np.float32))

    tok_of = []
    wt_of = []
    for e in range(NUM_EXPERTS):
        mask = sel == e  # [T, 2]
        toks = np.nonzero(mask.any(axis=1))[0]
        we = np.where(mask[toks, 0], w[toks, 0], w[toks, 1])
        tok_of.append(toks)
        wt_of.append(we.astype(np.float32))

    C_E = max(128, int(-(-counts.max() // 128)) * 128)
    C_S = T // N_CORES

    key = (C_E, C_S)
    if key not in _CACHE:
        _patch_tile_drain()
        _CACHE[key] = _build_moe_nc(C_E, C_S, TB=512)
    nc = _CACHE[key]

    xT_bf = np.ascontiguousarray(x.T).astype(BF16_NP)  # [D, T]

    in_maps = []
    for c in range(N_CORES):
        toks = tok_of[c]
        xT_e = np.zeros((D, C_E), dtype=BF16_NP)
        xT_e[:, :len(toks)] = xT_bf[:, toks]
        xT_s = np.ascontiguousarray(xT_bf[:, c * C_S:(c + 1) * C_S])
        in_maps.append({
            "xT_e": xT_e,
            "xT_s": xT_s,
            "wg_e": np.asarray(exp_gate[c], dtype=np.float32).astype(BF16_NP),
            "wu_e": np.asarray(exp_up[c], dtype=np.float32).astype(BF16_NP),
            "wd_e": np.asarray(exp_down[c], dtype=np.float32).astype(BF16_NP),
            "wg_s": np.asarray(shared_gate, dtype=np.float32).astype(BF16_NP),
            "wu_s": np.asarray(shared_up, dtype=np.float32).astype(BF16_NP),
            "wd_s": np.asarray(shared_down, dtype=np.float32).astype(BF16_NP),
        })

    trace = os.environ.get("MOE_TRACE") == "1"
    if trace:
        _ensure_ntff_hook()
    res = run_bass_kernel_spmd(nc, in_maps, core_ids=list(range(N_CORES)), trace=trace,
                               tmpdir=os.environ.get("MOE_TRACE_DIR") or None)
    kernel.last_exec_time_ns = res.exec_time_ns

    out = np.empty((T, Dm), dtype=np.float32)
    for c in range(N_CORES):
        out[c * C_S:(c + 1) * C_S] = res.results[c]["yT_s"].T
    for e in range(NUM_EXPERTS):
        toks = tok_of[e]
        y = res.results[e]["yT_e"][:, :len(toks)].T  # [n_e, D]
        out[toks] += wt_of[e][:, None] * y

    return out.reshape(B, S, Dm), np.float32(aux)
